# revision 1
# baseline (speedup 1.0000x reference)
"""MixConv depthwise conv (3x3/5x5/7x7 over 64-channel groups) as banded-Toeplitz
matmuls on the TensorEngine, sharded over 8 NeuronCores by channel.

Decomposition: a kxk depthwise conv = sum over dx of a 1D conv along H applied to
the input shifted by dx along W. The 1D conv along H is a matmul with a banded
[H, H] Toeplitz matrix (built host-side from the conv weights) contracting over
H=112 partitions. W-shifts are free-dim offsets into a padded SBUF image tile;
the dx-passes accumulate in PSUM.

Sharding: 192 channels / 8 cores = 24 channels per core, 8 from each kernel-size
group so PE work is balanced. Weights (Toeplitz form, ~6MB/core) ride along as an
extra input. Host stages x into the padded per-channel layout so every device DMA
is a dense 2D copy.

Matmuls run in fp32r (1 cycle/row vs fp32's 4): hardware RNE-rounds both
operands to 11-bit mantissa and accumulates exactly in fp32 PSUM — measured
~1.5e-4 scale-relative output error.
"""

import numpy as np

import concourse.bacc as bacc
import concourse.mybir as mybir
import concourse.tile as tile
from concourse.bass_utils import run_bass_kernel_spmd

# Problem constants (hardcoded per contract)
N_IMGS = 32
H = W = 112
GROUP_KS = (3, 5, 7)
GROUP_SIZE = 64          # channels per group
N_CORES = 8
CH_PER_GROUP_PER_CORE = GROUP_SIZE // N_CORES   # 8
CH_PER_CORE = CH_PER_GROUP_PER_CORE * len(GROUP_KS)  # 24

RW = W + 6               # per-image region width in the padded tile (max pad=3)
DATA_OFF = 3             # data cols at [3, 115) of each region
XCOLS = N_IMGS * RW + 8  # +8 slack for last-chunk matmul over-read
OCOLS = N_IMGS * W
N_MM = 4 * RW            # 472 — matmul free dim (4 images/chunk), even (fp32r)

KS = [3] * 8 + [5] * 8 + [7] * 8          # per-channel kernel size (core order)
TOFF = np.cumsum([0] + KS).tolist()       # tmat row offset per channel
N_TMAT = TOFF[-1]                          # 120

# "fp32": exact, 4 cyc/row.  "fp32r": 1 cyc/row, ~1.5e-4 rel err.
# "fp32r_split": weights split hi/lo, 2 fp32r passes — ~0.7e-4, 2 cyc/row.
MM_MODE = "fp32r"

_BASS_CACHE = {}


def _build_bass(mode, reps=1):
    nsplit = 2 if mode == "fp32r_split" else 1
    use_f32r = mode in ("fp32r", "fp32r_split")
    mm_dt = mybir.dt.float32r if use_f32r else mybir.dt.float32
    f32 = mybir.dt.float32

    nc = bacc.Bacc("TRN2", target_bir_lowering=False, debug=False)
    xp_d = nc.dram_tensor("xp", [CH_PER_CORE, H, XCOLS], f32, kind="ExternalInput")
    t_d = nc.dram_tensor("tmat", [nsplit * N_TMAT, H, H], f32, kind="ExternalInput")
    y_d = nc.dram_tensor("y", [CH_PER_CORE, H, OCOLS], f32, kind="ExternalOutput")

    def src(ap):
        return ap.bitcast(mm_dt) if use_f32r else ap

    with tile.TileContext(nc) as tc:
        with (
            tc.tile_pool(name="xpool", bufs=2) as xpool,
            tc.tile_pool(name="tpool", bufs=2) as tpool,
            tc.tile_pool(name="opool", bufs=2) as opool,
            tc.tile_pool(name="pspool", bufs=8, space="PSUM") as pspool,
        ):
            for rep in range(reps):
              for ch in range(CH_PER_CORE):
                k = KS[ch]
                pad = (k - 1) // 2
                x_t = xpool.tile([H, XCOLS], mm_dt, tag="x", name=f"x{rep}_{ch}")
                nc.sync.dma_start(x_t[:, :], src(xp_d[ch]))
                t_t = tpool.tile([H, nsplit * 7 * H], mm_dt, tag="t", name=f"t{rep}_{ch}")
                for s in range(nsplit):
                    nc.sync.dma_start(
                        t_t[:, s * k * H : (s + 1) * k * H].rearrange(
                            "p (d m) -> p d m", d=k
                        ),
                        src(
                            t_d[
                                s * N_TMAT + TOFF[ch] : s * N_TMAT + TOFF[ch] + k
                            ].rearrange("d hin hout -> hin d hout")
                        ),
                    )
                out_t = opool.tile([H, OCOLS], f32, tag="o", name=f"o{rep}_{ch}")
                passes = [(s, dx) for s in range(nsplit) for dx in range(k)]
                for half in range(2):
                    pts = [
                        pspool.tile(
                            [H, N_MM], f32, tag="ps", name=f"ps{rep}_{ch}_{half}_{b}"
                        )
                        for b in range(4)
                    ]
                    for pi, (s, dx) in enumerate(passes):
                        off = dx - pad + DATA_OFF
                        lhsT = t_t[:, (s * k + dx) * H : (s * k + dx + 1) * H]
                        for b in range(4):
                            base = (16 * half + 4 * b) * RW
                            nc.tensor.matmul(
                                pts[b],
                                lhsT=lhsT,
                                rhs=x_t[:, base + off : base + off + N_MM],
                                start=(pi == 0),
                                stop=(pi == len(passes) - 1),
                            )
                    for b in range(4):
                        img0 = 16 * half + 4 * b
                        nc.any.tensor_copy(
                            out=out_t.rearrange("p (i w) -> p i w", i=N_IMGS)[
                                :, img0 : img0 + 4, :
                            ],
                            in_=pts[b].rearrange("p (i r) -> p i r", i=4)[:, :, :W],
                        )
                nc.sync.dma_start(y_d[ch], out_t[:, :])
    nc.compile()
    return nc


def _get_bass(mode, reps=1):
    key = (mode, reps)
    if key not in _BASS_CACHE:
        _BASS_CACHE[key] = _build_bass(mode, reps)
    return _BASS_CACHE[key]


def _build_toeplitz(w, k):
    """w: [C, 1, k, k] -> T: [C, k, H, H], T[c,dx,hin,hout] = w[c,0,hin-hout+pad,dx]."""
    pad = (k - 1) // 2
    C = w.shape[0]
    T = np.zeros((C, k, H, H), np.float32)
    for dy in range(k):
        off = pad - dy  # hout = hin + off
        hin = np.arange(max(0, -off), H - max(0, off))
        T[:, :, hin, hin + off] = w[:, 0, dy, :][:, :, None]
    return T


def _round_fp32r(a):
    """RNE round fp32 to 11-bit mantissa (the fp32r grid) — matches HW."""
    u = a.astype(np.float32).view(np.uint32).astype(np.uint64)
    lsb = (u >> 12) & 1
    u = (u + 0x7FF + lsb) & 0xFFFFF000
    return u.astype(np.uint32).view(np.float32)


def _core_channels(core):
    out = []
    for g in range(len(GROUP_KS)):
        base = g * GROUP_SIZE + core * CH_PER_GROUP_PER_CORE
        out.extend(range(base, base + CH_PER_GROUP_PER_CORE))
    return out


def _prepare_in_maps(x, w3, w5, w7, mode):
    x = np.ascontiguousarray(np.asarray(x, dtype=np.float32))
    ws = {3: np.asarray(w3, np.float32), 5: np.asarray(w5, np.float32),
          7: np.asarray(w7, np.float32)}
    Ts = {k: _build_toeplitz(ws[k], k) for k in GROUP_KS}

    in_maps = []
    for core in range(N_CORES):
        chs = _core_channels(core)
        # staged x: [24, H, XCOLS], data at [i*RW+3, i*RW+115) per image
        xp = np.zeros((CH_PER_CORE, H, XCOLS), np.float32)
        xv = xp[:, :, : N_IMGS * RW].reshape(CH_PER_CORE, H, N_IMGS, RW)
        xv[:, :, :, DATA_OFF : DATA_OFF + W] = x[:, chs].transpose(1, 2, 0, 3)

        tm = np.concatenate(
            [
                Ts[GROUP_KS[g]][
                    core * CH_PER_GROUP_PER_CORE : (core + 1) * CH_PER_GROUP_PER_CORE
                ].reshape(-1, H, H)
                for g in range(len(GROUP_KS))
            ],
            axis=0,
        )
        assert tm.shape[0] == N_TMAT
        if mode == "fp32r_split":
            hi = _round_fp32r(tm)
            lo = tm - hi
            tm = np.concatenate([hi, lo], axis=0)
        in_maps.append({"xp": xp, "tmat": np.ascontiguousarray(tm)})
    return in_maps


def _gather(results):
    out = np.empty((N_IMGS, GROUP_SIZE * len(GROUP_KS), H, W), np.float32)
    for core in range(N_CORES):
        chs = _core_channels(core)
        y = results[core]["y"].reshape(CH_PER_CORE, H, N_IMGS, W)
        out[:, chs] = y.transpose(2, 0, 1, 3)
    return out


def run(x, w3, w5, w7, **spmd_kwargs):
    """Full run; returns (output, BassKernelResults) for profiling access."""
    nc = _get_bass(MM_MODE)
    in_maps = _prepare_in_maps(x, w3, w5, w7, MM_MODE)
    br = run_bass_kernel_spmd(nc, in_maps, core_ids=list(range(N_CORES)), **spmd_kwargs)
    return _gather(br.results), br


def kernel(x, w3, w5, w7):
    out, _ = run(x, w3, w5, w7)
    return out



# revision 2
# speedup vs baseline: 1.6248x; 1.6248x over previous
"""MixConv depthwise conv (3x3/5x5/7x7 over 64-channel groups) as banded-Toeplitz
matmuls on the TensorEngine, sharded over 8 NeuronCores by channel.

Decomposition: a kxk depthwise conv = sum over dx of a 1D conv along H applied to
the input shifted by dx along W. The 1D conv along H is a matmul with a banded
[H, H] Toeplitz matrix (built host-side from the conv weights) contracting over
H=112 partitions. W-shifts are free-dim offsets into a padded SBUF image tile;
the dx-passes accumulate in PSUM (8 banks = 8 x 4-image chunks per channel).

Sharding: 192 channels / 8 cores = 24 channels per core, 8 from each kernel-size
group so PE work is balanced.

All HBM traffic is bf16 (x, Toeplitz weights, y) — the kernel is near the
DMA/PE ridge, and fp32 staging makes it DMA-bound with the PE HAM-throttled.
bf16 rounding of x/w plus bf16 output storage measures ~3e-3 max rel err vs
the 2e-2 gate. PSUM accumulation stays fp32. The Toeplitz matrices for all 24
channels stay resident in SBUF (27 KB/partition), loaded as 3 dense DMAs.
"""

import numpy as np
import ml_dtypes

import concourse.bacc as bacc
import concourse.mybir as mybir
import concourse.tile as tile
from concourse.bass_utils import run_bass_kernel_spmd

# Problem constants (hardcoded per contract)
N_IMGS = 32
H = W = 112
GROUP_KS = (3, 5, 7)
GROUP_SIZE = 64          # channels per group
N_CORES = 8
CH_PER_GROUP_PER_CORE = GROUP_SIZE // N_CORES   # 8
CH_PER_CORE = CH_PER_GROUP_PER_CORE * len(GROUP_KS)  # 24

RW = W + 6               # per-image region width in the padded tile (max pad=3)
DATA_OFF = 3             # data cols at [3, 115) of each region
XCOLS = N_IMGS * RW + 8  # +8 slack for last-chunk matmul over-read
OCOLS = N_IMGS * W
N_MM = 4 * RW            # 472 — matmul free dim (4 images/chunk), one PSUM bank
N_BANKS = 8              # image chunks / PSUM banks per channel

KS = [3] * 8 + [5] * 8 + [7] * 8          # per-channel kernel size (core order)
TOFF = np.cumsum([0] + KS).tolist()       # tmat col-block offset per channel
N_TMAT = TOFF[-1]                          # 120 [H,H] Toeplitz slices
TCOLS = N_TMAT * H                         # 13440

MM_MODE = "bf16"

_BASS_CACHE = {}


def _build_bass():
    bf16 = mybir.dt.bfloat16
    f32 = mybir.dt.float32

    nc = bacc.Bacc("TRN2", target_bir_lowering=False, debug=False)
    xp_d = nc.dram_tensor("xp", [CH_PER_CORE, H, XCOLS], bf16, kind="ExternalInput")
    t_d = nc.dram_tensor("tmat", [H, TCOLS], bf16, kind="ExternalInput")
    y_d = nc.dram_tensor("y", [CH_PER_CORE, H, OCOLS], bf16, kind="ExternalOutput")

    with tile.TileContext(nc) as tc:
        with (
            tc.tile_pool(name="xpool", bufs=3) as xpool,
            tc.tile_pool(name="tpool", bufs=1) as tpool,
            tc.tile_pool(name="opool", bufs=3) as opool,
            tc.tile_pool(name="pspool", bufs=N_BANKS, space="PSUM") as pspool,
        ):
            t_t = tpool.tile([H, TCOLS], bf16, name="tmat")
            # one dense DMA per kernel-size group so ch 0 starts promptly
            for g in range(len(GROUP_KS)):
                c0 = TOFF[8 * g] * H
                c1 = TOFF[8 * (g + 1)] * H
                nc.sync.dma_start(t_t[:, c0:c1], t_d[:, c0:c1])

            for ch in range(CH_PER_CORE):
                k = KS[ch]
                pad = (k - 1) // 2
                x_t = xpool.tile([H, XCOLS], bf16, tag="x", name=f"x{ch}")
                nc.sync.dma_start(x_t[:, :], xp_d[ch])
                out_t = opool.tile([H, OCOLS], bf16, tag="o", name=f"o{ch}")
                pts = [
                    pspool.tile([H, N_MM], f32, tag="ps", name=f"ps{ch}_{b}")
                    for b in range(N_BANKS)
                ]
                for dx in range(k):
                    off = dx - pad + DATA_OFF
                    lhsT = t_t[:, (TOFF[ch] + dx) * H : (TOFF[ch] + dx + 1) * H]
                    for b in range(N_BANKS):
                        base = 4 * b * RW
                        nc.tensor.matmul(
                            pts[b],
                            lhsT=lhsT,
                            rhs=x_t[:, base + off : base + off + N_MM],
                            start=(dx == 0),
                            stop=(dx == k - 1),
                        )
                for b in range(N_BANKS):
                    img0 = 4 * b
                    nc.any.tensor_copy(
                        out=out_t.rearrange("p (i w) -> p i w", i=N_IMGS)[
                            :, img0 : img0 + 4, :
                        ],
                        in_=pts[b].rearrange("p (i r) -> p i r", i=4)[:, :, :W],
                    )
                nc.sync.dma_start(y_d[ch], out_t[:, :])
    nc.compile()
    return nc


def _get_bass():
    if "nc" not in _BASS_CACHE:
        _BASS_CACHE["nc"] = _build_bass()
    return _BASS_CACHE["nc"]


def _build_toeplitz(w, k):
    """w: [C, 1, k, k] -> T: [C, k, H, H], T[c,dx,hin,hout] = w[c,0,hin-hout+pad,dx]."""
    pad = (k - 1) // 2
    C = w.shape[0]
    T = np.zeros((C, k, H, H), np.float32)
    for dy in range(k):
        off = pad - dy  # hout = hin + off
        hin = np.arange(max(0, -off), H - max(0, off))
        T[:, :, hin, hin + off] = w[:, 0, dy, :][:, :, None]
    return T


def _core_channels(core):
    out = []
    for g in range(len(GROUP_KS)):
        base = g * GROUP_SIZE + core * CH_PER_GROUP_PER_CORE
        out.extend(range(base, base + CH_PER_GROUP_PER_CORE))
    return out


def _prepare_in_maps(x, w3, w5, w7):
    x = np.ascontiguousarray(np.asarray(x, dtype=np.float32))
    ws = {3: np.asarray(w3, np.float32), 5: np.asarray(w5, np.float32),
          7: np.asarray(w7, np.float32)}
    Ts = {k: _build_toeplitz(ws[k], k) for k in GROUP_KS}

    in_maps = []
    for core in range(N_CORES):
        chs = _core_channels(core)
        # staged x: [24, H, XCOLS] bf16, data at [i*RW+3, i*RW+115) per image
        xp = np.zeros((CH_PER_CORE, H, XCOLS), ml_dtypes.bfloat16)
        xv = xp[:, :, : N_IMGS * RW].reshape(CH_PER_CORE, H, N_IMGS, RW)
        xv[:, :, :, DATA_OFF : DATA_OFF + W] = x[:, chs].transpose(1, 2, 0, 3)

        # resident Toeplitz: [hin, (ch, dx, hout)] bf16
        tm = np.concatenate(
            [
                Ts[GROUP_KS[g]][
                    core * CH_PER_GROUP_PER_CORE : (core + 1) * CH_PER_GROUP_PER_CORE
                ].reshape(-1, H, H)
                for g in range(len(GROUP_KS))
            ],
            axis=0,
        )  # [120, hin, hout]
        assert tm.shape[0] == N_TMAT
        tmd = np.ascontiguousarray(
            tm.transpose(1, 0, 2).reshape(H, TCOLS)
        ).astype(ml_dtypes.bfloat16)
        in_maps.append({"xp": xp, "tmat": tmd})
    return in_maps


def _gather(results):
    out = np.empty((N_IMGS, GROUP_SIZE * len(GROUP_KS), H, W), np.float32)
    for core in range(N_CORES):
        chs = _core_channels(core)
        y = results[core]["y"].astype(np.float32).reshape(CH_PER_CORE, H, N_IMGS, W)
        out[:, chs] = y.transpose(2, 0, 1, 3)
    return out


def run(x, w3, w5, w7, **spmd_kwargs):
    """Full run; returns (output, BassKernelResults) for profiling access."""
    nc = _get_bass()
    in_maps = _prepare_in_maps(x, w3, w5, w7)
    br = run_bass_kernel_spmd(nc, in_maps, core_ids=list(range(N_CORES)), **spmd_kwargs)
    return _gather(br.results), br


def kernel(x, w3, w5, w7):
    out, _ = run(x, w3, w5, w7)
    return out


# revision 5
# speedup vs baseline: 1.6984x; 1.0453x over previous
"""MixConv depthwise conv (3x3/5x5/7x7 over 64-channel groups) as banded-Toeplitz
matmuls on the TensorEngine, sharded over 8 NeuronCores by channel.

Decomposition: a kxk depthwise conv = sum over dx of a 1D conv along H applied to
the input shifted by dx along W. The 1D conv along H is a matmul with a banded
[H, H] Toeplitz matrix (built host-side from the conv weights) contracting over
H=112 partitions. W-shifts are free-dim offsets into a padded SBUF image tile;
the dx-passes accumulate in PSUM. Matmul rhs uses a segmented AP ([4 images x
112 cols], stride 118) so the inter-image pad columns are never streamed.

Sharding: 192 channels / 8 cores = 24 channels per core, 8 from each kernel-size
group so PE work is balanced. Channels run k-descending (7,5,3) so DMA prefetch
builds headroom while the PE chews the big kernels.

All HBM traffic is bf16 (x, Toeplitz weights, y): the kernel sits on the DMA/PE
ridge and fp32 staging makes it DMA-bound with the PE HAM-throttled. PSUM
accumulation stays fp32; measured ~4e-3 max rel err vs the 2e-2 gate. Traffic is
split over three DMA paths (x loads on the SP HWDGE ring, y stores on the ACT
ring, Toeplitz tables on the gpsimd SWDGE path) and staged as channel-pair
transfers (1.7 MB, 15 KB/descriptor) to stay off the single-ring rate limit.
"""

import numpy as np
import ml_dtypes

import concourse.bacc as bacc
import concourse.mybir as mybir
import concourse.tile as tile
from concourse.bass_utils import run_bass_kernel_spmd

# Problem constants (hardcoded per contract)
N_IMGS = 32
H = W = 112
GROUP_KS = (7, 5, 3)     # device processing order: k-descending
GROUP_SIZE = 64          # channels per group
N_CORES = 8
CH_PER_GROUP_PER_CORE = GROUP_SIZE // N_CORES   # 8
CH_PER_CORE = CH_PER_GROUP_PER_CORE * len(GROUP_KS)  # 24
N_PAIRS = CH_PER_CORE // 2

RW = W + 6               # per-image region width in the padded tile (max pad=3)
DATA_OFF = 3             # data cols at [3, 115) of each region
XCOLS = N_IMGS * RW + 6  # 3782 — +6 so the last bank's 4*RW slice stays in range
OCOLS = N_IMGS * W       # 3584
N_BANKS = 8              # image chunks / PSUM banks per channel
IMG_PER_BANK = N_IMGS // N_BANKS  # 4
N_MM = IMG_PER_BANK * W  # 448 — matmul free size (segmented, pads skipped)

KS = [7] * 8 + [5] * 8 + [3] * 8          # per-channel kernel size (device order)
TOFF = np.cumsum([0] + KS).tolist()       # tmat col-block offset per channel
N_TMAT = TOFF[-1]                          # 120 [H,H] Toeplitz slices
TCOLS = N_TMAT * H                         # 13440

MM_MODE = "bf16"

_BASS_CACHE = {}


def _build_bass():
    bf16 = mybir.dt.bfloat16
    f32 = mybir.dt.float32

    nc = bacc.Bacc("TRN2", target_bir_lowering=False, debug=False)
    xp_d = nc.dram_tensor(
        "xp", [N_PAIRS, H, 2 * XCOLS], bf16, kind="ExternalInput"
    )
    t_d = nc.dram_tensor("tmat", [H, TCOLS], bf16, kind="ExternalInput")
    y_d = nc.dram_tensor(
        "y", [N_PAIRS, H, 2 * OCOLS], bf16, kind="ExternalOutput"
    )

    with tile.TileContext(nc) as tc:
        with (
            tc.tile_pool(name="xpool", bufs=3) as xpool,
            tc.tile_pool(name="tpool", bufs=1) as tpool,
            tc.tile_pool(name="opool", bufs=2) as opool,
            tc.tile_pool(name="pspool", bufs=N_BANKS, space="PSUM") as pspool,
        ):
            # Per-group Toeplitz tables (separate tiles so the first matmul
            # only waits on its own group's table), SWDGE path.
            t_tiles = []
            for g, kg in enumerate(GROUP_KS):
                c0 = TOFF[8 * g] * H
                c1 = TOFF[8 * (g + 1)] * H
                t_g = tpool.tile([H, c1 - c0], bf16, tag=f"t{kg}", name=f"t{kg}")
                nc.gpsimd.dma_start(t_g[:, :], t_d[:, c0:c1])
                t_tiles.append((t_g, c0))

            for pair in range(N_PAIRS):
                x_t = xpool.tile([H, 2 * XCOLS], bf16, tag="x", name=f"x{pair}")
                nc.sync.dma_start(x_t[:, :], xp_d[pair])
                out_t = opool.tile([H, 2 * OCOLS], bf16, tag="o", name=f"o{pair}")
                for c in range(2):
                    ch = 2 * pair + c
                    k = KS[ch]
                    pad = (k - 1) // 2
                    t_g, c0 = t_tiles[ch // 8]
                    for b in range(N_BANKS):
                        pt = pspool.tile([H, N_MM], f32, tag="ps", name=f"ps{ch}_{b}")
                        base = c * XCOLS + IMG_PER_BANK * b * RW
                        for dx in range(k):
                            off = dx - pad + DATA_OFF
                            tc0 = (TOFF[ch] + dx) * H - c0
                            nc.tensor.matmul(
                                pt,
                                lhsT=t_g[:, tc0 : tc0 + H],
                                rhs=x_t[
                                    :, base + off : base + off + IMG_PER_BANK * RW
                                ].rearrange("p (i r) -> p i r", i=IMG_PER_BANK)[
                                    :, :, :W
                                ],
                                start=(dx == 0),
                                stop=(dx == k - 1),
                            )
                        img0 = IMG_PER_BANK * b
                        nc.any.tensor_copy(
                            out=out_t.rearrange(
                                "p (c i w) -> p c i w", c=2, i=N_IMGS
                            )[:, c, img0 : img0 + IMG_PER_BANK, :],
                            in_=pt.rearrange("p (i w) -> p i w", i=IMG_PER_BANK),
                        )
                nc.scalar.dma_start(y_d[pair], out_t[:, :])
    nc.compile()
    return nc


def _get_bass():
    if "nc" not in _BASS_CACHE:
        _BASS_CACHE["nc"] = _build_bass()
    return _BASS_CACHE["nc"]


def _build_toeplitz(w, k):
    """w: [C, 1, k, k] -> T: [C, k, H, H], T[c,dx,hin,hout] = w[c,0,hin-hout+pad,dx]."""
    pad = (k - 1) // 2
    C = w.shape[0]
    T = np.zeros((C, k, H, H), np.float32)
    for dy in range(k):
        off = pad - dy  # hout = hin + off
        hin = np.arange(max(0, -off), H - max(0, off))
        T[:, :, hin, hin + off] = w[:, 0, dy, :][:, :, None]
    return T


def _core_channels(core):
    """Global channel ids for this core, in device (k-descending) order."""
    out = []
    for g, kg in enumerate(GROUP_KS):
        gidx = {3: 0, 5: 1, 7: 2}[kg]
        base = gidx * GROUP_SIZE + core * CH_PER_GROUP_PER_CORE
        out.extend(range(base, base + CH_PER_GROUP_PER_CORE))
    return out


def _prepare_in_maps(x, w3, w5, w7):
    x = np.ascontiguousarray(np.asarray(x, dtype=np.float32))
    ws = {3: np.asarray(w3, np.float32), 5: np.asarray(w5, np.float32),
          7: np.asarray(w7, np.float32)}
    Ts = {k: _build_toeplitz(ws[k], k) for k in GROUP_KS}

    in_maps = []
    for core in range(N_CORES):
        chs = _core_channels(core)
        # staged x: [pair, H, (c, img, RW)] bf16, data at [3, 115) per region,
        # +6 zero slack cols at the end of each channel region
        xs = np.zeros((N_PAIRS, H, 2, N_IMGS, RW), ml_dtypes.bfloat16)
        xc = x[:, chs]  # [N, 24, H, W]
        xs[:, :, :, :, DATA_OFF : DATA_OFF + W] = (
            xc.transpose(1, 2, 0, 3)          # [24, H, N, W]
            .reshape(N_PAIRS, 2, H, N_IMGS, W)
            .transpose(0, 2, 1, 3, 4)         # [12, H, 2, N, W]
        )
        xs = xs.reshape(N_PAIRS, H, 2, N_IMGS * RW)
        xp = np.zeros((N_PAIRS, H, 2, XCOLS), ml_dtypes.bfloat16)
        xp[:, :, :, : N_IMGS * RW] = xs
        xp = np.ascontiguousarray(xp.reshape(N_PAIRS, H, 2 * XCOLS))

        # resident Toeplitz: [hin, (ch, dx, hout)] bf16, device channel order
        tm = np.concatenate(
            [
                Ts[kg][
                    core * CH_PER_GROUP_PER_CORE : (core + 1) * CH_PER_GROUP_PER_CORE
                ].reshape(-1, H, H)
                for kg in GROUP_KS
            ],
            axis=0,
        )  # [120, hin, hout]
        assert tm.shape[0] == N_TMAT
        tmd = np.ascontiguousarray(
            tm.transpose(1, 0, 2).reshape(H, TCOLS)
        ).astype(ml_dtypes.bfloat16)
        in_maps.append({"xp": xp, "tmat": tmd})
    return in_maps


def _gather(results):
    out = np.empty((N_IMGS, GROUP_SIZE * len(GROUP_KS), H, W), np.float32)
    for core in range(N_CORES):
        chs = _core_channels(core)
        y = results[core]["y"].astype(np.float32)
        y = y.reshape(N_PAIRS, H, 2, N_IMGS, W).transpose(0, 2, 1, 3, 4)
        out[:, chs] = y.reshape(CH_PER_CORE, H, N_IMGS, W).transpose(2, 0, 1, 3)
    return out


def run(x, w3, w5, w7, **spmd_kwargs):
    """Full run; returns (output, BassKernelResults) for profiling access."""
    nc = _get_bass()
    in_maps = _prepare_in_maps(x, w3, w5, w7)
    br = run_bass_kernel_spmd(nc, in_maps, core_ids=list(range(N_CORES)), **spmd_kwargs)
    return _gather(br.results), br


def kernel(x, w3, w5, w7):
    out, _ = run(x, w3, w5, w7)
    return out


# revision 6
# speedup vs baseline: 1.8315x; 1.0783x over previous
"""MixConv depthwise conv (3x3/5x5/7x7 over 64-channel groups) as banded-Toeplitz
matmuls on the TensorEngine, sharded over 8 NeuronCores by channel.

Decomposition: a kxk depthwise conv = sum over dx of a 1D conv along H applied to
the input shifted by dx along W. The 1D conv along H is a matmul with a banded
[H, H] Toeplitz matrix (built host-side from the conv weights) contracting over
H=112 partitions. W-shifts are free-dim offsets into a padded SBUF image tile;
the dx-passes accumulate in PSUM. Matmul rhs uses a segmented AP ([4 images x
112 cols], stride 118) so the inter-image pad columns are never streamed; PE
runs at the 1 col/cycle bf16 roofline with ~197 ns/matmul pitch.

Sharding: 192 channels / 8 cores = 24 channels per core, 8 from each kernel-size
group so PE work is balanced. Channels run k-descending (7,5,3) so DMA prefetch
builds headroom while the PE chews the big kernels.

All HBM traffic is bf16 (x, Toeplitz weights, y): fp32 staging makes the kernel
DMA-bound with the PE HAM-throttled. PSUM accumulation stays fp32; measured
~4e-3 max rel err vs the 2e-2 gate. Latency shaping: x loads ride the SP HWDGE
ring as channel-pair transfers (first pair split so ch0 starts early), y stores
and the per-channel Toeplitz tables ride the ACT ring, and a short burst of
dependency-free warmup matmuls on a memset scratch tile keeps the PE busy and
HAM-warm while the first transfers land.
"""

import numpy as np
import ml_dtypes

import concourse.bacc as bacc
import concourse.mybir as mybir
import concourse.tile as tile
from concourse.bass_utils import run_bass_kernel_spmd

# Problem constants (hardcoded per contract)
N_IMGS = 32
H = W = 112
GROUP_KS = (7, 5, 3)     # device processing order: k-descending
GROUP_SIZE = 64          # channels per group
N_CORES = 8
CH_PER_GROUP_PER_CORE = GROUP_SIZE // N_CORES   # 8
CH_PER_CORE = CH_PER_GROUP_PER_CORE * len(GROUP_KS)  # 24
N_PAIRS = CH_PER_CORE // 2

RW = W + 6               # per-image region width in the padded tile (max pad=3)
DATA_OFF = 3             # data cols at [3, 115) of each region
XCOLS = N_IMGS * RW + 6  # 3782 — +6 so the last bank's 4*RW slice stays in range
OCOLS = N_IMGS * W       # 3584
N_BANKS = 8              # image chunks / PSUM banks per channel
IMG_PER_BANK = N_IMGS // N_BANKS  # 4
N_MM = IMG_PER_BANK * W  # 448 — matmul free size (segmented, pads skipped)
N_WARMUP = 20            # dep-free matmuls to keep PE busy+warm during head DMAs

KS = [7] * 8 + [5] * 8 + [3] * 8          # per-channel kernel size (device order)
TOFF = np.cumsum([0] + KS).tolist()       # tmat col-block offset per channel
N_TMAT = TOFF[-1]                          # 120 [H,H] Toeplitz slices
TCOLS = N_TMAT * H                         # 13440

MM_MODE = "bf16"

_BASS_CACHE = {}


def _build_bass():
    bf16 = mybir.dt.bfloat16
    f32 = mybir.dt.float32

    nc = bacc.Bacc("TRN2", target_bir_lowering=False, debug=False)
    xp_d = nc.dram_tensor(
        "xp", [N_PAIRS, H, 2 * XCOLS], bf16, kind="ExternalInput"
    )
    t_d = nc.dram_tensor("tmat", [H, TCOLS], bf16, kind="ExternalInput")
    y_d = nc.dram_tensor(
        "y", [CH_PER_CORE, H, OCOLS], bf16, kind="ExternalOutput"
    )

    with tile.TileContext(nc) as tc:
        with (
            tc.tile_pool(name="xpool", bufs=3) as xpool,
            tc.tile_pool(name="tpool", bufs=1) as tpool,
            tc.tile_pool(name="opool", bufs=3) as opool,
            tc.tile_pool(name="wpool", bufs=1) as wpool,
            tc.tile_pool(name="pspool", bufs=N_BANKS, space="PSUM") as pspool,
        ):
            # PE warmup: dep-free matmuls on a memset tile fill the initial
            # DMA wait and take the HAM clock-gate to 8/8 before real work.
            w_t = wpool.tile([H, N_MM], bf16, tag="warm", name="warm")
            nc.any.memset(w_t[:, :], 0.0)
            pw = pspool.tile([H, N_MM], f32, tag="ps", name="ps_warm")
            for i in range(N_WARMUP):
                nc.tensor.matmul(
                    pw, lhsT=w_t[:, :H], rhs=w_t[:, :], start=True, stop=True
                )

            # Per-channel Toeplitz tables on the ACT ring (idle until stores
            # begin); separate tiles keep the dependency per channel.
            t_tiles = []
            for ch in range(CH_PER_CORE):
                k = KS[ch]
                t_c = tpool.tile([H, k * H], bf16, tag=f"t{ch}", name=f"t{ch}")
                nc.scalar.dma_start(
                    t_c[:, :], t_d[:, TOFF[ch] * H : (TOFF[ch] + k) * H]
                )
                t_tiles.append(t_c)

            for pair in range(N_PAIRS):
                if pair == 0:
                    # split so ch0's matmuls start as early as possible
                    x_t = xpool.tile([H, 2 * XCOLS], bf16, tag="x", name="x0")
                    for c in range(2):
                        nc.sync.dma_start(
                            x_t[:, c * XCOLS : (c + 1) * XCOLS],
                            xp_d[0].rearrange("p (c w) -> p c w", c=2)[:, c, :],
                        )
                else:
                    x_t = xpool.tile([H, 2 * XCOLS], bf16, tag="x", name=f"x{pair}")
                    nc.sync.dma_start(x_t[:, :], xp_d[pair])
                for c in range(2):
                    ch = 2 * pair + c
                    k = KS[ch]
                    pad = (k - 1) // 2
                    t_c = t_tiles[ch]
                    out_t = opool.tile([H, OCOLS], bf16, tag="o", name=f"o{ch}")
                    for b in range(N_BANKS):
                        pt = pspool.tile([H, N_MM], f32, tag="ps", name=f"ps{ch}_{b}")
                        base = c * XCOLS + IMG_PER_BANK * b * RW
                        for dx in range(k):
                            off = dx - pad + DATA_OFF
                            nc.tensor.matmul(
                                pt,
                                lhsT=t_c[:, dx * H : (dx + 1) * H],
                                rhs=x_t[
                                    :, base + off : base + off + IMG_PER_BANK * RW
                                ].rearrange("p (i r) -> p i r", i=IMG_PER_BANK)[
                                    :, :, :W
                                ],
                                start=(dx == 0),
                                stop=(dx == k - 1),
                            )
                        img0 = IMG_PER_BANK * b
                        nc.any.tensor_copy(
                            out=out_t.rearrange("p (i w) -> p i w", i=N_IMGS)[
                                :, img0 : img0 + IMG_PER_BANK, :
                            ],
                            in_=pt.rearrange("p (i w) -> p i w", i=IMG_PER_BANK),
                        )
                    nc.scalar.dma_start(y_d[ch], out_t[:, :])
    nc.compile()
    return nc


def _get_bass():
    if "nc" not in _BASS_CACHE:
        _BASS_CACHE["nc"] = _build_bass()
    return _BASS_CACHE["nc"]


def _build_toeplitz(w, k):
    """w: [C, 1, k, k] -> T: [C, k, H, H], T[c,dx,hin,hout] = w[c,0,hin-hout+pad,dx]."""
    pad = (k - 1) // 2
    C = w.shape[0]
    T = np.zeros((C, k, H, H), np.float32)
    for dy in range(k):
        off = pad - dy  # hout = hin + off
        hin = np.arange(max(0, -off), H - max(0, off))
        T[:, :, hin, hin + off] = w[:, 0, dy, :][:, :, None]
    return T


def _core_channels(core):
    """Global channel ids for this core, in device (k-descending) order."""
    out = []
    for kg in GROUP_KS:
        gidx = {3: 0, 5: 1, 7: 2}[kg]
        base = gidx * GROUP_SIZE + core * CH_PER_GROUP_PER_CORE
        out.extend(range(base, base + CH_PER_GROUP_PER_CORE))
    return out


def _prepare_in_maps(x, w3, w5, w7):
    x = np.ascontiguousarray(np.asarray(x, dtype=np.float32))
    ws = {3: np.asarray(w3, np.float32), 5: np.asarray(w5, np.float32),
          7: np.asarray(w7, np.float32)}
    Ts = {k: _build_toeplitz(ws[k], k) for k in GROUP_KS}

    in_maps = []
    for core in range(N_CORES):
        chs = _core_channels(core)
        # staged x: [pair, H, (c, img, RW)] bf16, data at [3, 115) per region,
        # +6 zero slack cols at the end of each channel region
        xs = np.zeros((N_PAIRS, H, 2, N_IMGS, RW), ml_dtypes.bfloat16)
        xc = x[:, chs]  # [N, 24, H, W]
        xs[:, :, :, :, DATA_OFF : DATA_OFF + W] = (
            xc.transpose(1, 2, 0, 3)          # [24, H, N, W]
            .reshape(N_PAIRS, 2, H, N_IMGS, W)
            .transpose(0, 2, 1, 3, 4)         # [12, H, 2, N, W]
        )
        xs = xs.reshape(N_PAIRS, H, 2, N_IMGS * RW)
        xp = np.zeros((N_PAIRS, H, 2, XCOLS), ml_dtypes.bfloat16)
        xp[:, :, :, : N_IMGS * RW] = xs
        xp = np.ascontiguousarray(xp.reshape(N_PAIRS, H, 2 * XCOLS))

        # resident Toeplitz: [hin, (ch, dx, hout)] bf16, device channel order
        tm = np.concatenate(
            [
                Ts[kg][
                    core * CH_PER_GROUP_PER_CORE : (core + 1) * CH_PER_GROUP_PER_CORE
                ].reshape(-1, H, H)
                for kg in GROUP_KS
            ],
            axis=0,
        )  # [120, hin, hout]
        assert tm.shape[0] == N_TMAT
        tmd = np.ascontiguousarray(
            tm.transpose(1, 0, 2).reshape(H, TCOLS)
        ).astype(ml_dtypes.bfloat16)
        in_maps.append({"xp": xp, "tmat": tmd})
    return in_maps


def _gather(results):
    out = np.empty((N_IMGS, GROUP_SIZE * len(GROUP_KS), H, W), np.float32)
    for core in range(N_CORES):
        chs = _core_channels(core)
        y = results[core]["y"].astype(np.float32).reshape(CH_PER_CORE, H, N_IMGS, W)
        out[:, chs] = y.transpose(2, 0, 1, 3)
    return out


def run(x, w3, w5, w7, **spmd_kwargs):
    """Full run; returns (output, BassKernelResults) for profiling access."""
    nc = _get_bass()
    in_maps = _prepare_in_maps(x, w3, w5, w7)
    br = run_bass_kernel_spmd(nc, in_maps, core_ids=list(range(N_CORES)), **spmd_kwargs)
    return _gather(br.results), br


def kernel(x, w3, w5, w7):
    out, _ = run(x, w3, w5, w7)
    return out
